# revision 31
# baseline (speedup 1.0000x reference)
"""Multi-head attention (B=2, S=2048, D=1024, H=16) on 8 trn2 NeuronCores.

Sharding: head-parallel. Core c owns heads {2c, 2c+1} (= feature rows
[128c, 128c+128) of the QKV projections / columns of Wo). Each core:
  - projects full q/k/v (pre-transposed + cast on host) against its
    128-column slice of Wq/Wk/Wv,
  - runs softmax(QK^T * s) @ V for its 4 (batch, head) pairs using a
    transposed-score layout (keys on partitions) so no on-chip transposes
    are needed,
  - computes its additive partial of the output projection
    (attn_heads @ Wo[:, cols].T) in row-parallel fashion.
Host sums the 8 partials and adds the (bo + bv @ Wo.T) constant, which is
where the bv bias lands after the softmax-normalization algebra.

v8: partition-aligned packed layouts kill every partition-shift DMA and
the norm chain's four DRAM bounces:
  - K keeps both heads stacked on partitions (rows 0:64 = h0 dims,
    64:128 = h1), so one stationary K tile serves both heads' score
    matmuls; the per-head Q buffers zero the other head's rows instead.
  - V1 blocks are [V_h0 | ones | pad | V_h1] (192 cols/kt): the h1
    stationary slice [64:192] puts its denominator on PSUM row 0 and the
    AV output directly on rows 64:127, so both heads' attnT writes are
    partition-aligned.
  - softmax normalization = DVE reciprocal (bf16) + a 1-partition PE
    matmul against a host-built mask row that broadcasts the reciprocal
    over 64 partitions, + one DVE multiply. No DRAM round trips.
The ACT engine's 128 back-to-back 1us exps are the pace; projection
chunks c2..c7 and the output projection are spread across the attention
positions so the PE stays just under that pace, and the tail (last
chunk's fc) rides two alternating PSUM tags with DVE/ACT drains.
"""

import sys

for _p in ("/opt/trn_rl_repo",):
    if _p not in sys.path:
        try:
            import concourse  # noqa: F401
            break
        except ImportError:
            sys.path.insert(0, _p)

import numpy as np
import ml_dtypes

import concourse.bass as bass
import concourse.tile as tile
from concourse import mybir
from concourse.bass_utils import run_bass_kernel_spmd

BF16 = mybir.dt.bfloat16
F8 = mybir.dt.float8e4
F32 = mybir.dt.float32
AF = mybir.ActivationFunctionType
DR = mybir.MatmulPerfMode.DoubleRow

B, S, D, H, DH = 2, 2048, 1024, 16, 64
NCORES = 8
T = B * S              # 4096 tokens
HC = 128               # head-columns per core (2 heads x 64)
KO = D // 128          # 8 contraction tiles for projections
SCALE = DH ** -0.5     # 0.125
VB = 192               # V1 block: [V_h0 64 | ones 1 | pad 63 | V_h1 64]

_NC = None
_INLINE_NORM = False
_CHAIN_INLINE = False


def _split_multiwaits(nc, maxw=1):
    """Walrus codegen in this container rejects Drain instructions carrying
    more than ~2 semaphore waits ("Too many sync wait commands"). Move the
    excess waits onto preceding NoOps on the same engine."""
    ctr = 0
    for f in nc.m.functions:
        for bb in f.blocks:
            newlist = []
            changed = False
            for inst in bb.instructions:
                si = inst.sync_info
                if (si is not None and si.on_wait and len(si.on_wait) > maxw):
                    waits = list(si.on_wait)
                    for j in range(maxw, len(waits), maxw):
                        nop = mybir.InstNoOp(name=f"splitw-{ctr}", ins=[], outs=[])
                        ctr += 1
                        nop.engine = inst.engine
                        nop.sync_info = mybir.SyncInfo(
                            on_wait=list(waits[j:j + maxw]), on_update=[])
                        newlist.append(nop)
                    inst.sync_info = mybir.SyncInfo(
                        on_wait=waits[:maxw], on_update=list(si.on_update))
                    changed = True
                newlist.append(inst)
            if changed:
                bb.instructions = newlist
    return ctr


def _build(split=True):
    nc = bass.Bass()

    qT = nc.declare_dram_parameter("qT", [D, T], F8, isOutput=False)
    kT = nc.declare_dram_parameter("kT", [D, T], F8, isOutput=False)
    vT = nc.declare_dram_parameter("vT", [D, T], F8, isOutput=False)
    wq = nc.declare_dram_parameter("wq", [D, HC], F8, isOutput=False)
    wk = nc.declare_dram_parameter("wk", [D, HC], F8, isOutput=False)
    wv = nc.declare_dram_parameter("wv", [D, HC], F8, isOutput=False)
    bq = nc.declare_dram_parameter("bq", [HC, 1], F32, isOutput=False)
    bk = nc.declare_dram_parameter("bk", [HC, 1], F32, isOutput=False)
    wo = nc.declare_dram_parameter("wo", [HC, D], BF16, isOutput=False)
    ident = nc.declare_dram_parameter("ident", [128, 128], F32, isOutput=False)
    nmask = nc.declare_dram_parameter("nmask", [128, 128], BF16, isOutput=False)
    out = nc.declare_dram_parameter("out", [T, D], BF16, isOutput=True)

    qT3 = qT.rearrange("(ko p) n -> p ko n", p=128)
    kT3 = kT.rearrange("(ko p) n -> p ko n", p=128)
    vT3 = vT.rearrange("(ko p) n -> p ko n", p=128)
    wq3 = wq.rearrange("(ko p) m -> p ko m", p=128)
    wk3 = wk.rearrange("(ko p) m -> p ko m", p=128)
    wv3 = wv.rearrange("(ko p) m -> p ko m", p=128)

    NCH = T // 512       # 8 projection chunks of 512 tokens
    CHUNKS = [(b, qc) for b in range(B) for qc in range(2)]
    STEPS = [(kt, h) for h in range(2) for kt in range(16)]
    NPOS = len(CHUNKS) * 32          # 128 global score positions
    # Projection work is spread over three consecutive positions per chunk
    # (QK matmuls / V matmul / transposes+V1 copies) so the PE never inserts
    # a >2us lump between score matmuls. c2/c3 feed chunk-0 scores kt8/kt12;
    # c4..c7 are b=1, needed from pos 64 (c7's V is parked at 61/62 to keep
    # the fc PSUM ring free through chunk j1's fc tiles).
    PQ_AT = {6: 2, 10: 3, 20: 4, 28: 5, 38: 6, 52: 7}
    PK_AT = {7: 2, 11: 3, 21: 4, 29: 5, 39: 6, 53: 7}
    PV_AT = {8: 2, 12: 3, 22: 4, 30: 5, 40: 6, 61: 7}
    PVC_AT = {9: 2, 13: 3, 23: 4, 31: 5, 41: 6, 62: 7}
    # V fetch for chunk c kicked at this position (>=8 positions of lead)
    VK_AT = {4: 3, 14: 4, 22: 5, 33: 6, 53: 7}
    # q/k fetches staggered by need time so the startup-critical c0/c1
    # data doesn't share DMA bandwidth with 12MB of later fetches
    QK_AT = {2: 1, 10: 2, 26: 3}
    # chunk j-1's 8 fc tiles at these in-chunk positions of chunk j (late
    # enough that the previous chunk's h1 norm chain has landed in attnT)
    FC_AT = {14 + 2 * t: t for t in range(8)}

    with tile.TileContext(nc) as tc:
        with (
            tc.tile_pool(name="consts", bufs=1) as consts,
            tc.tile_pool(name="big", bufs=1) as big,
            tc.tile_pool(name="qkin", bufs=3) as qkin,
            tc.tile_pool(name="vin", bufs=3) as vin,
            tc.tile_pool(name="small", bufs=2) as small,
            tc.tile_pool(name="osp", bufs=4) as osp,
            tc.tile_pool(name="exps", bufs=7) as exps,
            tc.tile_pool(name="scp", bufs=2, space="PSUM") as scp,
            tc.tile_pool(name="fcp", bufs=1, space="PSUM") as fcp,
            tc.tile_pool(name="avp", bufs=1, space="PSUM") as avp,
            tc.tile_pool(name="dnm", bufs=2, space="DRAM") as dnm,
        ):
            # ---- persistent SBUF state ----
            wq_s = consts.tile([128, KO, 128], F8, tag="wq")
            wk_s = consts.tile([128, KO, 128], F8, tag="wk")
            wv_s = consts.tile([128, KO, 128], F8, tag="wv")
            wo_s = consts.tile([HC, D], BF16, tag="wo")
            bq_s = consts.tile([HC, 1], F32, tag="bq")
            bk_s = consts.tile([HC, 1], F32, tag="bk")
            id_s = consts.tile([128, 128], F32, tag="id")
            nm_s = consts.tile([128, 128], BF16, tag="nm")

            # Zero-padded per-head Q (rows of the other head zeroed); K keeps
            # both heads stacked (the moving Q zeros select the head).
            QTp = [big.tile([128, T], BF16, tag=f"QTp{h}", name=f"QTp{h}")
                   for h in range(2)]
            KTpk = big.tile([128, T], BF16, tag="KTpk")
            attnT = big.tile([HC, T], BF16, tag="attnT")
            V1 = [big.tile([128, 16 * VB], BF16, tag=f"V1_{b}", name=f"V1_{b}")
                  for b in range(B)]

            # ---------------- emission helpers ----------------

            qk_stage = {}
            v_stage = {}

            def emit_qk_dmas(cpair, first=False):
                """Fetch q/k for chunk pair cpair (1024 tokens). The first
                fetch keeps SP's kick count low (DGE queue credits stall SP
                past ~6 outstanding kicks) and puts consts on the gp queue;
                later fetches ride gp so mid-kernel SP stays free for the
                norm chains."""
                q2 = qkin.tile([128, KO, 1024], F8, tag="q_in")
                k2 = qkin.tile([128, KO, 1024], F8, tag="k_in")
                cs2 = bass.ds(cpair * 1024, 1024)
                if first:
                    nc.sync.dma_start(q2[:, 0:2, :], qT3[:, 0:2, cs2])
                    nc.sync.dma_start(wk_s[:], wk3[:])
                    nc.sync.dma_start(k2[:, 0:2, :], kT3[:, 0:2, cs2])
                    nc.sync.dma_start(q2[:, 2:KO, :], qT3[:, 2:KO, cs2])
                    nc.sync.dma_start(k2[:, 2:KO, :], kT3[:, 2:KO, cs2])
                    nc.gpsimd.dma_start(bq_s[:], bq[:])
                    nc.gpsimd.dma_start(bk_s[:], bk[:])
                    nc.gpsimd.dma_start(wv_s[:], wv3[:])
                    nc.gpsimd.dma_start(id_s[:], ident[:])
                    nc.gpsimd.dma_start(nm_s[:], nmask[:])
                else:
                    nc.gpsimd.dma_start(q2[:], qT3[:, :, cs2])
                    nc.gpsimd.dma_start(k2[:], kT3[:, :, cs2])
                qk_stage[cpair] = (q2, k2)

            def emit_v_dma(c):
                v_in = vin.tile([128, KO, 512], F8, tag="v_in")
                nc.gpsimd.dma_start(v_in[:], vT3[:, :, bass.ts(c, 512)])
                v_stage[c] = v_in

            psv_stage = {}
            pqk_stage = {}

            def emit_proj_q(c):
                """Q projection matmuls + bias adds for 512-token chunk c.
                PSUM rides the score ring (one slot spans the Q and K
                parts: Q in cols 0:512, K in 512:1024)."""
                cs = bass.ts(c, 512)
                q2, _ = qk_stage[c // 2]
                off = (c % 2) * 512
                ps = scp.tile([128, 1024], F32, tag="sp", name=f"psqk{c}")
                for m in range(KO // 2):
                    nc.tensor.matmul(ps[:, 0:512],
                                     wq_s[:, 2 * m: 2 * m + 2, :],
                                     q2[:, 2 * m: 2 * m + 2, off:off + 512],
                                     start=(m == 0), stop=(m == KO // 2 - 1),
                                     perf_mode=DR)
                nc.vector.tensor_scalar_add(QTp[0][0:64, cs], ps[0:64, 0:512],
                                            bq_s[0:64, 0:1])
                nc.vector.tensor_scalar_add(QTp[1][64:128, cs],
                                            ps[64:128, 0:512],
                                            bq_s[64:128, 0:1])
                pqk_stage[c] = ps

            def emit_proj_k(c):
                cs = bass.ts(c, 512)
                _, k2 = qk_stage[c // 2]
                off = (c % 2) * 512
                ps = pqk_stage.pop(c)
                for m in range(KO // 2):
                    nc.tensor.matmul(ps[:, 512:1024],
                                     wk_s[:, 2 * m: 2 * m + 2, :],
                                     k2[:, 2 * m: 2 * m + 2, off:off + 512],
                                     start=(m == 0), stop=(m == KO // 2 - 1),
                                     perf_mode=DR)
                nc.vector.tensor_scalar_add(KTpk[:, cs], ps[:, 512:1024],
                                            bk_s[:, 0:1])

            def emit_proj_v(c):
                """V projection for chunk c: wv-stationary into cols 0:512
                of an fc-ring slot, staged to SBUF for the transposes.
                No bias (bv folds into the host const)."""
                v_in = v_stage[c]
                ps_v = fcp.tile([128, 1024], F32, tag="fp", name=f"psv{c}")
                for m in range(KO // 2):
                    nc.tensor.matmul(ps_v[:, 0:512],
                                     wv_s[:, 2 * m: 2 * m + 2, :],
                                     v_in[:, 2 * m: 2 * m + 2, :],
                                     start=(m == 0), stop=(m == KO // 2 - 1),
                                     perf_mode=DR)
                vts = small.tile([128, 512], F32, tag="vts")
                nc.vector.tensor_copy(vts[:], ps_v[:, 0:512])
                psv_stage[c] = (ps_v, vts)

            def emit_proj_vtc(c):
                """PE identity-transposes into cols 512:1024 of the same
                slot, then per-head slices into the V1 blocks."""
                ps_v, vts = psv_stage.pop(c)
                for sub in range(4):
                    nc.tensor.matmul(ps_v[:, bass.ds(512 + sub * 128, 128)],
                                     vts[:, bass.ts(sub, 128)], id_s[:],
                                     start=True, stop=True,
                                     is_transpose=True)
                for sub in range(4):
                    tok0 = c * 512 + sub * 128
                    b, kt = tok0 // S, (tok0 % S) // 128
                    src = 512 + sub * 128
                    nc.vector.tensor_copy(
                        V1[b][:, kt * VB: kt * VB + 64],
                        ps_v[:, src: src + 64])
                    nc.vector.tensor_copy(
                        V1[b][:, kt * VB + 128: kt * VB + 192],
                        ps_v[:, src + 64: src + 128])

            e_ring = {}           # (j, kt, h) -> e tile
            av_cur = {}           # j -> [av0, av1]
            avs_cur = {}          # (j, h) -> avs sbuf drain
            rcp_cur = {}          # (j, h) -> rcp tile

            def emit_scores_step(j, kt, h):
                b, qc = CHUNKS[j]
                q0 = b * S + qc * 1024
                sp = scp.tile([128, 1024], F32, tag="sp", name=f"sp{j}_{kt}_{h}")
                for half in range(2):
                    nc.tensor.matmul(
                        sp[:, bass.ts(half, 512)],
                        KTpk[:, bass.ds(b * S + kt * 128, 128)],
                        QTp[h][:, bass.ds(q0 + half * 512, 512)],
                        start=True, stop=True)
                et = exps.tile([128, 1024], BF16, tag="et", name=f"et{j}_{kt}_{h}")
                nc.scalar.activation(et[:], sp[:], AF.Exp, scale=SCALE)
                e_ring[(j, kt, h)] = et

            def emit_av_step(j, kt, h):
                av = av_cur.setdefault(j, [None, None])
                b, qc = CHUNKS[j]
                if kt == 0:
                    av[h] = avp.tile([128, 1024], F32, tag="av",
                                     name=f"av{j}_{h}")
                et = e_ring.pop((j, kt, h))
                # h0 stationary [V_h0 | ones] -> AV rows 0:64, denom row 64;
                # h1 stationary [ones | pad | V_h1] -> denom row 0, AV rows
                # 64:128 (partition-aligned with attnT's h1 rows).
                if h == 0:
                    stat = V1[b][:, kt * VB: kt * VB + 65]
                else:
                    stat = V1[b][:, kt * VB + 64: kt * VB + 192]
                for half in range(2):
                    o = (av[h][0:65, bass.ts(half, 512)] if h == 0
                         else av[h][:, bass.ts(half, 512)])
                    nc.tensor.matmul(o, stat, et[:, bass.ts(half, 512)],
                                     start=(kt == 0), stop=(kt == 15))

            def emit_norm_drain(j, h):
                """As soon as head h of chunk j's AV retires: drain it to
                SBUF (frees the PSUM accumulator for the next head) and
                launch the denominator row toward [64,16] via DRAM (the only
                way to re-partition; a [1,1024] DVE reciprocal is ~8us).
                Kicks ride the otherwise-idle SP queue."""
                av = av_cur[j][h]
                avs = small.tile([128, 1024], F32, tag="avs")
                if h == 0:
                    nc.vector.tensor_copy(avs[0:65, :], av[0:65, :])
                else:
                    # rows 1:63 are zero (pad cols), one copy is cheaper
                    # than two partial ones
                    nc.vector.tensor_copy(avs[:, :], av[:, :])
                r = 64 if h == 0 else 0
                dscr = dnm.tile([1, 1024], F32, tag="dscr")
                nc.sync.dma_start(dscr[:], avs[r:r + 1, :])
                dsp = small.tile([64, 16], F32, tag="dsp")
                nc.sync.dma_start(
                    dsp[:], dscr.rearrange("o (p j) -> (o p) j", p=64))
                avs_cur[(j, h)] = avs
                rcp_cur[(j, h)] = dsp

            def emit_norm_recip(j, h):
                """Lane-parallel reciprocal of the respread denominators,
                then send them back out to DRAM for the broadcast read."""
                dsp = rcp_cur[(j, h)]
                rsp = small.tile([64, 16], F32, tag="rsp")
                nc.vector.reciprocal(rsp[:], dsp[:])
                dscr2 = dnm.tile([1, 1024], F32, tag="dscr2")
                nc.sync.dma_start(
                    dscr2.rearrange("o (p j) -> (o p) j", p=64), rsp[:])
                # land the broadcast on the head's own partitions so the
                # multiply's SBUF operands share a base partition
                lo = 0 if h == 0 else 64
                bcs = small.tile([128, 1024], F32, tag="bcs")
                nc.sync.dma_start(
                    bcs[lo:lo + 64, :],
                    dscr2[0:1, :].to_broadcast((64, 1024)))
                rcp_cur[(j, h)] = bcs

            def emit_norm_mul(j, h):
                b, qc = CHUNKS[j]
                q0 = b * S + qc * 1024
                avs = avs_cur.pop((j, h))
                bcs = rcp_cur.pop((j, h))
                lo = 0 if h == 0 else 64
                nc.vector.tensor_mul(attnT[lo:lo + 64, bass.ds(q0, 1024)],
                                     avs[lo:lo + 64, :],
                                     bcs[lo:lo + 64, :])

            def emit_norm_tail_half(j, h, half, state):
                """Last head, one 512-query half: reciprocal directly on the
                denominator row half (DVE is idle at the tail), 1-partition
                PE broadcast against the host mask row, multiply. Splitting
                in half lets half-0's fc tiles overlap half-1's reciprocal."""
                b, qc = CHUNKS[j]
                q0 = b * S + qc * 1024
                av = av_cur[j][h]
                r = 64 if h == 0 else 0
                lo = 0 if h == 0 else 64
                hs = bass.ts(half, 512)
                if half == 0:
                    avs = small.tile([128, 1024], F32, tag="avs")
                    rcp = small.tile([128, 1024], BF16, tag="rcp")
                    bc = scp.tile([128, 1024], F32, tag="sp",
                                  name=f"bct{j}_{h}")
                    state.update(avs=avs, rcp=rcp, bc=bc)
                    nc.vector.tensor_copy(avs[:, :], av[:, :])
                else:
                    avs, rcp, bc = state["avs"], state["rcp"], state["bc"]
                with nc.allow_low_precision("softmax denom reciprocal"):
                    nc.vector.reciprocal(rcp[r:r + 1, hs], avs[r:r + 1, hs])
                nc.tensor.matmul(bc[:, hs], nm_s[r:r + 1, :],
                                 rcp[r:r + 1, hs], start=True, stop=True)
                nc.vector.tensor_mul(
                    attnT[lo:lo + 64, bass.ds(q0 + half * 512, 512)],
                    avs[lo:lo + 64, hs], bc[lo:lo + 64, hs])

            def emit_fc_tile(j, tt, ring=None, drain="v"):
                """One 128-token tile of the output projection for chunk j."""
                b, qc = CHUNKS[j]
                t0 = b * S + qc * 1024 + tt * 128
                pool = ring if ring is not None else fcp
                tag = "fp" if pool is fcp else "sp"
                fp = pool.tile([128, 1024], F32, tag=tag, name=f"fp{j}_{tt}")
                for half in range(2):
                    hs = bass.ts(half, 512)
                    nc.tensor.matmul(fp[:, hs],
                                     attnT[:, bass.ds(t0, 128)],
                                     wo_s[:, hs], start=True, stop=True)
                os_ = osp.tile([128, 1024], BF16, tag="os")
                if drain == "v":
                    nc.vector.tensor_copy(os_[:], fp[:])
                else:
                    nc.scalar.copy(os_[:], fp[:])
                nc.gpsimd.dma_start(out[bass.ds(t0, 128), :], os_[:])

            # AV trails its score step by 2 positions normally; give the
            # first two steps after a head/chunk switch 2 extra positions
            # so the retiring head's PSUM drain isn't on the PE's critical
            # path.
            av_pending = []
            norm_pending = []

            def av_due(p):
                s = p % 32
                extra = 2 if (s in (0, 1) and p >= 32) or s in (16, 17) else 0
                return p + 2 + extra

            def emit_attn_pos(p, flush=False):
                # pipelined norm stages: recip 4 positions after the drain
                # (DMA respread round trip), mul 4 more (broadcast readback)
                while norm_pending and norm_pending[0][0] <= p:
                    _, stage, ja, ha = norm_pending.pop(0)
                    if stage == "recip":
                        emit_norm_recip(ja, ha)
                        norm_pending.append((p + 4, "mul", ja, ha))
                    else:
                        emit_norm_mul(ja, ha)
                if p < NPOS:
                    j, s = divmod(p, 32)
                    emit_scores_step(j, *STEPS[s])
                    av_pending.append(p)
                    if s in FC_AT and j >= 1:
                        emit_fc_tile(j - 1, FC_AT[s])
                while av_pending and (flush or av_due(av_pending[0]) <= p):
                    pa = av_pending.pop(0)
                    ja, sa = divmod(pa, 32)
                    kt, h = STEPS[sa]
                    emit_av_step(ja, kt, h)
                    if kt == 15:
                        if pa == NPOS - 1:
                            st = {}
                            emit_norm_tail_half(ja, h, 0, st)
                            for tt in range(4):
                                emit_fc_tile(ja, tt,
                                             ring=(fcp if tt % 2 == 0
                                                   else scp),
                                             drain="s")
                            emit_norm_tail_half(ja, h, 1, st)
                            for tt in range(4, 8):
                                emit_fc_tile(ja, tt,
                                             ring=(fcp if tt % 2 == 0
                                                   else scp),
                                             drain=("s" if tt < 6 else "v"))
                        elif _CHAIN_INLINE:
                            emit_norm_drain(ja, h)
                            emit_norm_recip(ja, h)
                            emit_norm_mul(ja, h)
                        else:
                            emit_norm_drain(ja, h)
                            norm_pending.append((p + 4, "recip", ja, h))

            # ---------------- emission schedule ----------------
            # gp: the memset gating the first scores, then V fetches
            nc.gpsimd.memset(QTp[0][64:128, 0:2048], 0.0)
            emit_v_dma(0)
            emit_v_dma(1)
            nc.sync.dma_start(wq_s[:], wq3[:])
            emit_qk_dmas(0, first=True)
            nc.gpsimd.memset(QTp[1][0:64, 0:2048], 0.0)
            # ones column at block col 64, pad cols 65:128 zeroed
            for b in range(B):
                v3 = V1[b].rearrange("p (t s) -> p t s", s=VB)
                nc.gpsimd.memset(v3[:, :, 64:128], 0.0)
                nc.gpsimd.memset(v3[:, :, 64], 1.0)

            emit_v_dma(2)
            emit_proj_q(0)
            emit_proj_k(0)
            emit_proj_v(0)
            emit_proj_vtc(0)
            emit_proj_q(1)
            emit_proj_k(1)
            emit_proj_v(1)
            emit_proj_vtc(1)

            wo_kicked = False
            for pos in range(NPOS):
                if pos == 16:
                    nc.gpsimd.memset(QTp[0][64:128, 2048:4096], 0.0)
                    nc.gpsimd.memset(QTp[1][0:64, 2048:4096], 0.0)
                if pos in QK_AT:
                    emit_qk_dmas(QK_AT[pos])
                if pos in VK_AT:
                    emit_v_dma(VK_AT[pos])
                if pos in PQ_AT:
                    emit_proj_q(PQ_AT[pos])
                if pos in PK_AT:
                    emit_proj_k(PK_AT[pos])
                if pos in PV_AT:
                    emit_proj_v(PV_AT[pos])
                if pos in PVC_AT:
                    emit_proj_vtc(PVC_AT[pos])
                    if not wo_kicked and pos >= 30:
                        nc.sync.dma_start(wo_s[:], wo[:])
                        wo_kicked = True
                emit_attn_pos(pos)
            # drain the AV queue + remaining norm stages + last norm
            p = NPOS
            while av_pending or norm_pending:
                emit_attn_pos(p, flush=True)
                p += 1


    if split:
        _split_multiwaits(nc)
    return nc


def _get_nc():
    global _NC
    if _NC is None:
        _NC = _build()
    return _NC


def _prep_in_maps(q, k, v, Wq, bq, Wk, bk, Wv, bv, Wo, bo):
    bf = ml_dtypes.bfloat16
    f8 = ml_dtypes.float8_e4m3
    qT = np.ascontiguousarray(q.reshape(T, D).T).astype(f8)
    kT = np.ascontiguousarray(k.reshape(T, D).T).astype(f8)
    vT = np.ascontiguousarray(v.reshape(T, D).T).astype(f8)
    nmask = np.zeros((128, 128), bf)
    nmask[64, 0:64] = 1.0   # h0: broadcast onto partitions 0..63
    nmask[0, 64:128] = 1.0  # h1: broadcast onto partitions 64..127
    in_maps = []
    for c in range(NCORES):
        rows = slice(c * HC, (c + 1) * HC)
        in_maps.append({
            "qT": qT, "kT": kT, "vT": vT,
            "wq": np.ascontiguousarray(Wq[rows, :].T).astype(f8),
            "wk": np.ascontiguousarray(Wk[rows, :].T).astype(f8),
            "wv": np.ascontiguousarray(Wv[rows, :].T).astype(f8),
            "bq": np.ascontiguousarray(bq[rows]).astype(np.float32).reshape(HC, 1),
            "bk": np.ascontiguousarray(bk[rows]).astype(np.float32).reshape(HC, 1),
            "wo": np.ascontiguousarray(Wo[:, rows].T).astype(bf),
            "ident": np.eye(128, dtype=np.float32),
            "nmask": nmask,
        })
    return in_maps


def _run(inputs, trace=False):
    inputs = {k_: np.asarray(v_) for k_, v_ in inputs.items()}
    nc = _get_nc()
    in_maps = _prep_in_maps(**inputs)
    res = run_bass_kernel_spmd(nc, in_maps, core_ids=list(range(NCORES)),
                               trace=trace)
    acc = np.zeros((T, D), np.float64)
    for c in range(NCORES):
        acc += res.results[c]["out"].astype(np.float64)
    const = (inputs["bo"].astype(np.float64)
             + inputs["bv"].astype(np.float64) @ inputs["Wo"].astype(np.float64).T)
    acc += const[None, :]
    return acc.reshape(B, S, D).astype(np.float32), res


def kernel(**inputs) -> np.ndarray:
    return _run(inputs)[0]
